# revision 8
# baseline (speedup 1.0000x reference)
"""Trainium2 Bass kernel: 3D trilinear grid_sample with strict-bounds masking
(nn_DenseMotionNetwork). Data-parallel over batch N=8 -> 8 NeuronCores.

Host-side layout prep (per core/batch element):
  - table (32768, 384) bf16: row r = z*2048 + y*32 + q holds the
    (2z x 2y x 3x) x 32-channel patch anchored at (z, y, 2q); within a row the
    layout is elem[c*12 + t] with t = zz*6 + yy*3 + xx.
  - gxs/gys/gzs (128, 512) f32: grid coords with point i at [i % 128, i // 128].
  - ident (128, 128) f32: identity for PE-transpose.

Device (per core): compute gather rows + 12 interpolation coefficients from the
grid; one dma_gather row per point; DVE multiply/fold combine; PE transpose to
channel-major; DMA out. Output (32, 65536) f32 per core.
"""
import numpy as np

C = 32
ID, IH, IW = 16, 64, 64
N_CORES = 8
P = ID * IH * IW                     # 65536 points per batch element
NROWS = ID * IH * (IW // 2)          # 32768 table rows
ELEM = C * 12                        # 384 bf16 values per row

_COMPILED = {}


def _build(P_=P, CH=1024, PW_SPLIT=4, debug_dump=False):
    import concourse.bass as bass
    import concourse.bacc as bacc
    import concourse.mybir as mybir
    from concourse.mybir import AluOpType as ALU
    from concourse.mybir import ActivationFunctionType as ACTF
    from concourse.tile import TileContext

    FP32 = mybir.dt.float32
    BF16 = mybir.dt.bfloat16
    I16 = mybir.dt.int16
    I32 = mybir.dt.int32

    assert P_ % CH == 0 and CH % 1024 == 0
    NCOL = P_ // 128
    NCH = P_ // CH
    CB = CH // 128

    nc = bacc.Bacc("TRN2", debug=False)
    table = nc.dram_tensor("table", [NROWS, ELEM], BF16, kind="ExternalInput")
    gxs = nc.dram_tensor("gxs", [128, NCOL], FP32, kind="ExternalInput")
    gys = nc.dram_tensor("gys", [128, NCOL], FP32, kind="ExternalInput")
    gzs = nc.dram_tensor("gzs", [128, NCOL], FP32, kind="ExternalInput")
    ident = nc.dram_tensor("ident", [128, 128], FP32, kind="ExternalInput")
    out32 = nc.dram_tensor("out32", [C, P_], FP32, kind="ExternalOutput")
    NCOL0 = P_ // 128
    if debug_dump:
        dbg_gidx = nc.dram_tensor("dbg_gidx", [128, NCOL0 * 8], mybir.dt.int16,
                                  kind="ExternalOutput")
        dbg_coef = nc.dram_tensor("dbg_coef", [128, NCOL0 * 12], BF16,
                                  kind="ExternalOutput")
        dbg_g = nc.dram_tensor("dbg_g", [128, (CH // 128) * ELEM], BF16,
                               kind="ExternalOutput")

    with TileContext(nc) as tc:
        with (
            tc.tile_pool(name="persist", bufs=1) as persist,
            tc.tile_pool(name="pw", bufs=1) as pw,
            tc.tile_pool(name="gather", bufs=3) as gpool,
            tc.tile_pool(name="combine", bufs=2) as cpool,
            tc.tile_pool(name="psum", bufs=3, space="PSUM") as ppool,
        ):
            gx_sb = persist.tile([128, NCOL], FP32, name="gx_sb")
            gy_sb = persist.tile([128, NCOL], FP32, name="gy_sb")
            gz_sb = persist.tile([128, NCOL], FP32, name="gz_sb")
            id_sb = persist.tile([128, 128], FP32, name="id_sb")
            nc.sync.dma_start(gx_sb[:, :], gxs.ap())
            nc.sync.dma_start(gy_sb[:, :], gys.ap())
            nc.sync.dma_start(gz_sb[:, :], gzs.ap())
            nc.sync.dma_start(id_sb[:, :], ident.ap())

            coef_b = persist.tile([128, NCOL, 12], BF16, name="coef_b")
            gidx = persist.tile([128, NCOL * 8], I16, name="gidx")
            gidx3 = gidx.rearrange("p (w e) -> p w e", e=8)
            # the gather reads only partitions 0-15; zero once so the full
            # [128, *] idx AP is defined
            nc.vector.memset(gidx[:, :], 0)

            W = NCOL // PW_SPLIT
            for s in range(PW_SPLIT):
                sl = slice(s * W, (s + 1) * W)

                def t(nm):
                    return pw.tile([128, W], FP32, name=nm, tag=nm)

                # unnormalize: i = (g+1)*S - 0.5
                ix, iy, iz = t("ix"), t("iy"), t("iz")
                nc.scalar.activation(ix[:, :], gx_sb[:, sl], ACTF.Copy,
                                     bias=31.5, scale=32.0)
                nc.scalar.activation(iy[:, :], gy_sb[:, sl], ACTF.Copy,
                                     bias=31.5, scale=32.0)
                nc.scalar.activation(iz[:, :], gz_sb[:, sl], ACTF.Copy,
                                     bias=7.5, scale=8.0)

                # fractional / integer split. floor(x) via int32 roundtrip:
                # r = f32(i32(x)); floor = r - (r > x)  (exact for any
                # conversion rounding mode)
                fx, fy, fz = t("fx"), t("fy"), t("fz")
                x0, y0, z0 = t("x0"), t("y0"), t("z0")
                icvt = pw.tile([128, W], I32, name="icvt", tag="icvt")
                rf, gtt = t("rf"), t("gtt")

                def floor_to(dst, frac, src):
                    nc.vector.tensor_copy(icvt[:, :], src[:, :])
                    nc.vector.tensor_copy(rf[:, :], icvt[:, :])
                    nc.vector.tensor_tensor(gtt[:, :], rf[:, :], src[:, :],
                                            ALU.is_gt)
                    nc.vector.tensor_tensor(dst[:, :], rf[:, :], gtt[:, :],
                                            ALU.subtract)
                    nc.vector.tensor_tensor(frac[:, :], src[:, :], dst[:, :],
                                            ALU.subtract)

                floor_to(x0, fx, ix)
                floor_to(y0, fy, iy)
                floor_to(z0, fz, iz)

                # strict-bounds masks, folded into the 1D weights:
                # w0 = (1-f)*[(v>0)&(v<HI)],  w1 = f*[(v+1>0)&(v+1<HI)]
                wpair = {}
                for nm, v0, f, hi in (("x", x0, fx, 64.0), ("y", y0, fy, 64.0),
                                      ("z", z0, fz, 16.0)):
                    ma, mb = t("ma"), t("mb")
                    m0, m1 = t("m0"), t("m1")
                    nc.vector.tensor_scalar(ma[:, :], v0[:, :], 0.0, None,
                                            ALU.is_gt)
                    nc.vector.tensor_scalar(mb[:, :], v0[:, :], hi, None,
                                            ALU.is_lt)
                    nc.vector.tensor_tensor(m0[:, :], ma[:, :], mb[:, :], ALU.mult)
                    nc.vector.tensor_scalar(ma[:, :], v0[:, :], -0.5, None,
                                            ALU.is_gt)
                    nc.vector.tensor_scalar(mb[:, :], v0[:, :], hi - 1.0, None,
                                            ALU.is_lt)
                    nc.vector.tensor_tensor(m1[:, :], ma[:, :], mb[:, :], ALU.mult)
                    w0, w1, fneg = t(nm + "w0"), t(nm + "w1"), t("fn")
                    nc.vector.tensor_scalar(fneg[:, :], f[:, :], -1.0, 1.0,
                                            ALU.mult, ALU.add)
                    nc.vector.tensor_tensor(w0[:, :], fneg[:, :], m0[:, :],
                                            ALU.mult)
                    nc.vector.tensor_tensor(w1[:, :], f[:, :], m1[:, :], ALU.mult)
                    wpair[nm] = (w0, w1)

                xc, yc, zc = t("xc"), t("yc"), t("zc")
                nc.vector.tensor_scalar(xc[:, :], x0[:, :], 0.0, 63.0, ALU.max,
                                        ALU.min)
                nc.vector.tensor_scalar(yc[:, :], y0[:, :], 0.0, 63.0, ALU.max,
                                        ALU.min)
                nc.vector.tensor_scalar(zc[:, :], z0[:, :], 0.0, 15.0, ALU.max,
                                        ALU.min)

                # q = floor(xc/2); o = xc - 2q; row = zc*2048 + yc*32 + q
                o, hlf, q, row = t("o"), t("hlf"), t("q"), t("row")
                nc.vector.tensor_scalar(hlf[:, :], xc[:, :], 0.5, None, ALU.mult)
                nc.vector.tensor_copy(icvt[:, :], hlf[:, :])
                nc.vector.tensor_copy(rf[:, :], icvt[:, :])
                nc.vector.tensor_tensor(gtt[:, :], rf[:, :], hlf[:, :], ALU.is_gt)
                nc.vector.tensor_tensor(q[:, :], rf[:, :], gtt[:, :],
                                        ALU.subtract)
                nc.vector.scalar_tensor_tensor(o[:, :], q[:, :], -2.0, xc[:, :],
                                               ALU.mult, ALU.add)
                nc.vector.scalar_tensor_tensor(row[:, :], yc[:, :], 32.0, q[:, :],
                                               ALU.mult, ALU.add)
                nc.vector.scalar_tensor_tensor(row[:, :], zc[:, :], 2048.0,
                                               row[:, :], ALU.mult, ALU.add)
                idx16 = pw.tile([128, W], I16, name="idx16", tag="idx16")
                nc.vector.tensor_copy(idx16[:, :], row[:, :])

                # x coefficients over the 3-wide patch (slots o and o+1 live)
                wx0, wx1 = wpair["x"]
                oc, a2, c2 = t("ocm"), t("a2"), t("c2")
                nc.vector.tensor_scalar(oc[:, :], o[:, :], -1.0, 1.0, ALU.mult,
                                        ALU.add)
                cx0, cx1, cx2 = t("cx0"), t("cx1"), t("cx2")
                nc.vector.tensor_tensor(cx0[:, :], oc[:, :], wx0[:, :], ALU.mult)
                nc.vector.tensor_tensor(a2[:, :], o[:, :], wx0[:, :], ALU.mult)
                nc.vector.tensor_tensor(c2[:, :], oc[:, :], wx1[:, :], ALU.mult)
                nc.vector.tensor_tensor(cx1[:, :], a2[:, :], c2[:, :], ALU.add)
                nc.vector.tensor_tensor(cx2[:, :], o[:, :], wx1[:, :], ALU.mult)

                # coef12[t] = wz[zz]*wy[yy]*cx[xx], t = zz*6 + yy*3 + xx
                coef_f = pw.tile([128, W, 12], FP32, name="coef_f", tag="coef_f")
                wy0, wy1 = wpair["y"]
                wz0, wz1 = wpair["z"]
                for zz, wzv in ((0, wz0), (1, wz1)):
                    for yy, wyv in ((0, wy0), (1, wy1)):
                        czy = t("czy")
                        nc.vector.tensor_tensor(czy[:, :], wzv[:, :], wyv[:, :],
                                                ALU.mult)
                        for xx, cxv in ((0, cx0), (1, cx1), (2, cx2)):
                            nc.vector.tensor_tensor(
                                coef_f[:, :, zz * 6 + yy * 3 + xx],
                                czy[:, :], cxv[:, :], ALU.mult)
                nc.vector.tensor_copy(coef_b[:, sl, :], coef_f[:, :, :])

                # idx into the 16-partition-wrapped gather layout via DMA
                # (compute engines can't address partition offsets % 32):
                # gidx[plo, col*8 + phi] = idx16[phi*16 + plo, col]
                for phi in range(8):
                    nc.sync.dma_start(gidx3[0:16, sl, phi],
                                      idx16[16 * phi:16 * (phi + 1), :])

            # the gather ucode reads indices from each gpsimd core's own
            # 16-partition group: replicate rows 0-15 to groups 1-7
            for g in range(1, 8):
                nc.sync.dma_start(gidx[16 * g:16 * (g + 1), :], gidx[0:16, :])

            if debug_dump:
                nc.sync.dma_start(dbg_gidx.ap(), gidx[:, :])
                nc.sync.dma_start(dbg_coef.ap(),
                                  coef_b.rearrange("p w t -> p (w t)"))
            for k in range(NCH):
                G = gpool.tile([128, CB, C, 12], BF16, name="G", tag="G")
                nc.gpsimd.dma_gather(
                    G.rearrange("p b c t -> p b (c t)"),
                    table.ap(),
                    gidx[:, k * CB * 8:(k + 1) * CB * 8],
                    CH, CH, ELEM,
                )

                if debug_dump and k == 0:
                    nc.sync.dma_start(dbg_g.ap(),
                                      G.rearrange("p b c t -> p (b c t)"))
                coef_sl = coef_b[:, k * CB:(k + 1) * CB, :]
                coef_bc = coef_sl.unsqueeze(2).broadcast_to((128, CB, C, 12))

                P12 = cpool.tile([128, CB, C, 12], BF16, name="P12", tag="fold")
                nc.vector.tensor_tensor(P12[:, :, :, :], G[:, :, :, :], coef_bc,
                                        ALU.mult)
                F6 = cpool.tile([128, CB, C, 6], BF16, name="F6", tag="fold")
                nc.vector.tensor_tensor(F6[:, :, :, :], P12[:, :, :, 0:6],
                                        P12[:, :, :, 6:12], ALU.add)
                F3 = cpool.tile([128, CB, C, 3], BF16, name="F3", tag="fold")
                nc.vector.tensor_tensor(F3[:, :, :, :], F6[:, :, :, 0:3],
                                        F6[:, :, :, 3:6], ALU.add)
                F1 = cpool.tile([128, CB, C], FP32, name="F1", tag="foldf")
                nc.vector.tensor_tensor(F1[:, :, :], F3[:, :, :, 0],
                                        F3[:, :, :, 1], ALU.add)
                Fo = cpool.tile([128, CB, C], FP32, name="Fo", tag="Fo")
                nc.vector.tensor_tensor(Fo[:, :, :], F1[:, :, :],
                                        F3[:, :, :, 2], ALU.add)

                # PE transpose to channel-major; 8 blocks per 2-bank psum tile
                for g in range(CB // 8):
                    pt = ppool.tile([C, 8, 128], FP32, name="pt", tag="pt")
                    for j in range(8):
                        nc.tensor.transpose(pt[:, j, :], Fo[:, g * 8 + j, :],
                                            id_sb[:, :])
                    ost = cpool.tile([C, 1024], FP32, name="ost", tag="ost")
                    nc.scalar.copy(ost[:, :], pt.rearrange("c a p -> c (a p)"))
                    col0 = k * CH + g * 1024
                    nc.sync.dma_start(out32.ap()[:, col0:col0 + 1024],
                                      ost[:, :])
    nc.compile()
    return nc


def _prep_core_inputs(inp_n, grid_n):
    """inp_n (C, ID, IH, IW) f32, grid_n (D, H, W, 3) f32 -> input dict."""
    import ml_dtypes
    from numpy.lib.stride_tricks import as_strided
    bf16 = ml_dtypes.bfloat16
    volp = np.zeros((C, ID + 1, IH + 1, IW + 2), dtype=bf16)
    volp[:, :ID, :IH, :IW] = inp_n.astype(bf16)
    sC, sZ, sY, sX = volp.strides
    patches = as_strided(
        volp,
        shape=(C, ID, IH, IW // 2, 2, 2, 3),
        strides=(sC, sZ, sY, 2 * sX, sZ, sY, sX),
    )
    table = np.ascontiguousarray(
        patches.transpose(1, 2, 3, 0, 4, 5, 6)).reshape(NROWS, ELEM)

    g = grid_n.reshape(-1, 3).astype(np.float32)
    NCOL = P // 128
    return {
        "table": table,
        "gxs": np.ascontiguousarray(g[:, 0].reshape(NCOL, 128).T),
        "gys": np.ascontiguousarray(g[:, 1].reshape(NCOL, 128).T),
        "gzs": np.ascontiguousarray(g[:, 2].reshape(NCOL, 128).T),
        "ident": np.eye(128, dtype=np.float32),
    }


def _get_compiled(key="default"):
    if key not in _COMPILED:
        _COMPILED[key] = _build()
    return _COMPILED[key]


def _run(inputs, trace=False, core_ids=None):
    """Returns (output (8,32,16,64,64) f32, BassKernelResults)."""
    from concourse import bass_utils

    inp = np.asarray(inputs["input"], dtype=np.float32)
    grid = np.asarray(inputs["grid"], dtype=np.float32)
    ac = int(np.asarray(inputs["align_corners"]))
    assert ac == 0, "kernel specialized for align_corners=0"
    N = inp.shape[0]
    if core_ids is None:
        core_ids = list(range(N))

    nc = _get_compiled()
    in_maps = [_prep_core_inputs(inp[n], grid[n]) for n in range(len(core_ids))]
    res = bass_utils.run_bass_kernel_spmd(nc, in_maps, core_ids=core_ids,
                                          trace=trace)
    out = np.empty((len(core_ids), C, ID, IH, IW), dtype=np.float32)
    for n in range(len(core_ids)):
        out[n] = res.results[n]["out32"].reshape(C, ID, IH, IW)
    return out, res


def kernel(**inputs):
    out, _ = _run(inputs, trace=False)
    return out
